# revision 1
# baseline (speedup 1.0000x reference)
"""Trainium2 Bass kernel for nn_ExteriorDerivative (d of a 2-form via central FD).

Math: the reference's central finite difference collapses analytically:
  (x +/- eps e_d) @ W1 = z +/- eps*W1[d]  with z = x @ W1, and
  sin(z+a) - sin(z-a) = 2 cos(z) sin(a), so
  fd[d] = cos(z) @ (diag(sin(eps*W1[d])/eps) @ W2)
and the whole gather/sign/scatter pipeline folds into one (32, 35) matrix G:
  out = cos(x @ W1) @ G = g1 + (sin(z/2)^2) @ (-2 G),   g1 = G.sum(0)
using cos(z) = 1 - 2 sin^2(z/2)  (|z/2| < pi here, no range reduction).

Device pipeline per core (pure batch-parallel across 8 cores; 32768
samples/core packed 4 subgroups x 8192 columns so every elementwise tile
uses all 128 partitions; engine time in this regime is per *column*):
  mm1:  z = blockdiag(W1 x4)^T @ xt            [128, W] PSUM f32 (f16 mm)
  s   = Sin(0.5 z)                             ACT, PSUM->SBUF f16
  q   = s*s                                    DVE tensor_tensor (2x mode)
  mm2A: T1 = GA^T @ q                          [128, 512] PSUM
        GA [128,128] = blockdiag(-2G x3) plus rows 96:128 -> outputs 0:23
        of subgroup 3, so T1 rows are 3 full samples + 23/35 of a 4th.
  copy: och = T1 + g1 (per-partition f32 bias) DVE tensor_scalar / ACT Ident
        (GPSIMD cannot access PSUM, so copies split DVE/ACT only; ACT
        copies are deferred one group so they never stall the sin spine)
  mm2B: leftover 12 outputs of subgroup 3 via ldweights-stationary matmuls:
        T2[128 samp, 12] += q[96:128, blk]^T @ (-2G)[:, 23:35], with a
        rank-1 ones x g1[23:35] prefill matmul providing the bias.
  copy: o2 = T2 (pure DVE copy), DMA out       ot [128, 8960] f16
Timeline-shape choices (tuned against the TimelineSim cost model): group
widths 512 at the head/tail, 1024 mid; stores launch from Pool SWDGE early
and SP late (keeps SP.SEQ free for loads; avoids head-of-line blocking); a
t=0 dummy matmul starts the PE pstate-ramp clock so real matmuls run at
full rate; tiny first x chunk via Pool SWDGE overlaps the weight-blob load
on SP/HWDGE; input x shipped f16; weights in one f16 blob.
"""
import numpy as np
from itertools import combinations

DIM = 7
EPS = 1e-4
NCORES = 8
B = 262144
B_CORE = B // NCORES          # 32768
SUB = 4                       # subgroups stacked on partitions
COLS = B_CORE // SUB          # 8192 columns per core
K_IN = SUB * DIM              # 28 input partitions
GROUP = 1024                  # columns per z tile (2 psum banks)
PAIR = 2 * GROUP              # columns per s/q tile and per load/store
NPAIR = COLS // PAIR          # 4
NGRP = COLS // GROUP          # 8
T2_BLK = 128                  # samples per mm2B matmul (stationary free dim)
NT2 = COLS // T2_BLK          # 64 blocks
# T2 tile sizes in blocks (max 42 = 504 f32 cols per psum bank); a smaller
# final tile keeps the tail-critical copy short
T2_TILES = [32, 32]
T2_PER_TILE = max(T2_TILES)
T2_W = T2_PER_TILE * 12
assert sum(T2_TILES) == NT2
T2_STARTS = [sum(T2_TILES[:i]) for i in range(len(T2_TILES))]
def _t2_pos(tbi):
    for t in range(len(T2_TILES) - 1, -1, -1):
        if tbi >= T2_STARTS[t]:
            return t, tbi - T2_STARTS[t]
OUT_W = COLS + NT2 * 12       # 8960 output columns

# blob column layout (f16, [128, BW])
BC_W1 = 0                     # [0:28, 0:128]     blockdiag W1 x4
BC_GA = 128                   # [0:128, 128:256]  GA
BC_G2 = 256                   # [96:128, 256:268] (-2G)[:,23:35], partitions 96..127
BC_ONES = 268                 # [0:1, 268:396]    ones row (prefill lhsT)
BC_G1T2 = 396                 # [0:1, ...]        g1[23:35] tiled (prefill rhs)
BW = 396 + T2_W

# T1-copy engine schedule (16 chunks of 512): 'd' DVE / 'a' ACT (deferred)
COPY_SCHED = "dddaadddaddaadda"
POOL_SQ = set()               # Pool squares always lose (measured); keep empty

# ---- static exterior-derivative index maps (mirrors reference.py) ----
_IDX3 = list(combinations(range(DIM), 3))
_POS2 = {t: i for i, t in enumerate(combinations(range(DIM), 2))}
_D2 = []
for _out, (i, j, k) in enumerate(_IDX3):
    for _p, (a, b, c) in enumerate([(i, j, k), (j, i, k), (k, i, j)]):
        bc = tuple(sorted((b, c)))
        s = (-1) ** _p * (1 if (b, c) == bc else -1)
        _D2.append((_out, _POS2[bc], a, s))


def _build_G(W1: np.ndarray, W2: np.ndarray) -> np.ndarray:
    """G[j, o] = sum_t SIGNS[t] * sin(EPS*W1[DCOORD[t], j])/EPS * W2[j, IN_POS[t]]  (fp64)."""
    W1d = W1.astype(np.float64)
    W2d = W2.astype(np.float64)
    G = np.zeros((32, 35), dtype=np.float64)
    for out_pos, in_pos, dcoord, sign in _D2:
        G[:, out_pos] += sign * (np.sin(EPS * W1d[dcoord, :]) / EPS) * W2d[:, in_pos]
    return G


_PROG = None


def _get_prog():
    global _PROG
    if _PROG is not None:
        return _PROG
    import concourse.bacc as bacc
    import concourse.bass as bass
    import concourse.tile as tile
    import concourse.mybir as mybir
    from concourse.alu_op_type import AluOpType as Alu

    F32 = mybir.dt.float32
    F16 = mybir.dt.float16
    Sin = mybir.ActivationFunctionType.Sin
    Ident = mybir.ActivationFunctionType.Identity

    nc = bacc.Bacc("TRN2", target_bir_lowering=False, debug=False, num_devices=NCORES)
    xt = nc.dram_tensor("xt", [K_IN, COLS], F16, kind="ExternalInput")
    blob = nc.dram_tensor("blob", [128, BW], F16, kind="ExternalInput")
    g1f32 = nc.dram_tensor("g1f32", [128, 1], F32, kind="ExternalInput")
    ot = nc.dram_tensor("ot", [128, OUT_W], F16, kind="ExternalOutput")

    with tile.TileContext(nc) as tc:
        with (
            tc.tile_pool(name="singles", bufs=1) as singles,
            tc.tile_pool(name="xin", bufs=2) as xpool,
            tc.tile_pool(name="och", bufs=2) as opool,
            tc.tile_pool(name="o2ch", bufs=2) as o2pool,
            tc.tile_pool(name="ssp", bufs=2) as spool,
            tc.tile_pool(name="qqp", bufs=2) as qpool,
            tc.tile_pool(name="zps", bufs=2, space=bass.MemorySpace.PSUM) as zpsum,
            tc.tile_pool(name="t1ps", bufs=3, space=bass.MemorySpace.PSUM) as t1psum,
            tc.tile_pool(name="t2ps", bufs=1, space=bass.MemorySpace.PSUM) as t2psum,
        ):
            warm = singles.tile([1, 64], F16)
            nc.vector.memset(warm[:], 0.0)
            bl = singles.tile([128, BW], F16)
            nc.sync.dma_start(bl[0:K_IN, 0:128], blob[0:K_IN, 0:128])
            g1s = singles.tile([128, 1], F32)

            w1_ap = bl[0:K_IN, BC_W1:BC_W1 + 128]
            ga_ap = bl[0:128, BC_GA:BC_GA + 128]
            g2_ap = bl[96:128, BC_G2:BC_G2 + 12]
            ones_ap = bl[0:1, BC_ONES:BC_ONES + 128]
            g1t2_ap = bl[0:1, BC_G1T2:BC_G1T2 + T2_W]

            wps = t1psum.tile([128, 512], F32, tag="t1")
            nc.tensor.matmul(wps[0:1, 0:64], warm[0:1, 0:1], warm[0:1, 0:64])

            # group widths: short head groups (fast pipeline fill) and short
            # tail groups (short drain chain)
            widths = [512, 512, 1024, 1024, 1024, 1024, 1024, 1024, 512, 512]
            assert sum(widths) == COLS
            t2 = None
            tb = 0               # T2 block counter
            chunk = 0            # T1-copy chunk counter (16 total)
            pending_act = []     # deferred ACT copies (dst, t1) from prior group
            pending_store = []   # stores deferred until the prior group's
                                 # copies (incl. deferred ACT ones) are emitted
            pending_store2 = []  # two groups late (pool-squared groups)
            deferred_out = None  # out-stage of a Pool-squared group
            xins = {}
            ochs = {}
            sss = {}
            qqs = {}
            ngrp = len(widths)

            # leftover 12 outputs of subgroup 3 (q-stationary matmuls);
            # block -> tile mapping is fixed by the block index; accumulation
            # order within a tile is free (disjoint 12-col ranges)
            def t2_blocks(qq, po, W, tb0):
                nonlocal t2
                for b in range(W // T2_BLK):
                    tbi = tb0 + b
                    t, w = _t2_pos(tbi)
                    tw = T2_TILES[t] * 12
                    if w == 0:
                        t2 = t2psum.tile([128, T2_W], F32, tag="t2")
                        nc.tensor.matmul(t2[:, :tw], ones_ap, g1t2_ap[:, :tw],
                                         start=True, stop=False)
                    qb = qq[96:128, po + b * T2_BLK:po + (b + 1) * T2_BLK]
                    nc.tensor.matmul(t2[:, 12 * w:12 * w + 12], qb, g2_ap,
                                     start=False, stop=(w == T2_TILES[t] - 1),
                                     tile_position=(96, 0))
                    if w == T2_TILES[t] - 1:
                        o2 = o2pool.tile([128, T2_W], F16, tag="o2")
                        t2o = COLS + 12 * T2_STARTS[t]
                        nc.vector.tensor_copy(o2[:, :tw], t2[:, :tw])
                        (nc.scalar if tbi == NT2 - 1 else nc.gpsimd
                         ).dma_start(ot[:, t2o:t2o + tw], o2[:, :tw])

            # T1 copies (mm2A + psum->sbuf+bias) then the T2 side stream
            def out_stage(qq, och, po, W, tb0, chunk0, last, force_d):
                for i, s in enumerate(range(0, W, 512)):
                    cw = min(512, W - s)
                    t1 = t1psum.tile([128, 512], F32, tag="t1")
                    nc.tensor.matmul(t1[:, :cw], ga_ap,
                                     qq[:, po + s:po + s + cw])
                    dst = och[:, po + s:po + s + cw]
                    e = 'd' if force_d else COPY_SCHED[(chunk0 + i)
                                                       % len(COPY_SCHED)]
                    if e == 'a' and not last:
                        pending_act.append((dst, t1, cw))
                    elif e == 'a':
                        nc.scalar.activation(dst, t1[:, :cw], Ident,
                                             bias=g1s[:], scale=1.0)
                    else:
                        nc.vector.tensor_scalar(dst, t1[:, :cw], g1s[:], None,
                                                Alu.add)
                t2_blocks(qq, po, W, tb0)

            c0 = 0
            for g, W in enumerate(widths):
                pi = c0 // PAIR
                po = c0 - pi * PAIR
                if pi not in xins:
                    xin = xpool.tile([K_IN, PAIR], F16, tag="xin")
                    if pi == 0:
                        # tiny first chunk via Pool SWDGE overlapping the
                        # w1 load on SP/HWDGE -> earliest possible mm1;
                        # weight-blob remainder + bias column follow the
                        # second x chunk so they don't delay sin(g1)
                        nc.gpsimd.dma_start(xin[:, :512], xt[:, :512])
                        nc.sync.dma_start(xin[:, 512:], xt[:, 512:PAIR])
                        nc.sync.dma_start(bl[:, 128:], blob[:, 128:])
                        nc.sync.dma_start(g1s[:], g1f32[:])
                    else:
                        nc.sync.dma_start(xin[:], xt[:, pi * PAIR:(pi + 1) * PAIR])
                    xins[pi] = xin
                    och = opool.tile([128, PAIR], F16, tag="och")
                    ss = spool.tile([128, PAIR], F16, tag="ss")
                    qq = qpool.tile([128, PAIR], F16, tag="qq")
                    ochs[pi], sss[pi], qqs[pi] = och, ss, qq
                xin, och, ss, qq = xins[pi], ochs[pi], sss[pi], qqs[pi]
                last = g == ngrp - 1

                zp = zpsum.tile([128, GROUP], F32, tag="zp")
                for s in range(0, W, 512):
                    cw = min(512, W - s)
                    nc.tensor.matmul(zp[:, s:s + cw], w1_ap,
                                     xin[:, po + s:po + s + cw])
                nc.scalar.activation(ss[:, po:po + W], zp[:, :W], Sin,
                                     bias=0.0, scale=0.5)
                # deferred ACT copies sit after this sin in the ACT queue, so
                # their mm2A dependency is long satisfied -> no ACT stall
                for dst, t1p, cw in pending_act:
                    nc.scalar.activation(dst, t1p[:, :cw], Ident,
                                         bias=g1s[:], scale=1.0)
                pending_act = []
                for eng, *dma_args in pending_store:
                    eng.dma_start(*dma_args)
                pending_store = pending_store2
                pending_store2 = []
                sq_pool = (g in POOL_SQ and not last
                           and g + 1 not in POOL_SQ and g >= 1)
                defer = sq_pool and not last
                (nc.gpsimd if sq_pool else nc.vector).tensor_tensor(
                    qq[:, po:po + W], ss[:, po:po + W], ss[:, po:po + W],
                    Alu.mult)

                my_tb, my_chunk = tb, chunk
                tb += W // T2_BLK
                chunk += (W + 511) // 512
                if deferred_out is not None:
                    out_stage(*deferred_out)
                    deferred_out = None
                if defer:
                    # software-pipelined emission: this group's out-stage is
                    # emitted after the NEXT group's mm1/sin/sq, so mm1(g+1)
                    # never queues behind mm2A/T2 matmuls in the in-order PE
                    # stream and the copies' mm2A deps are met on arrival
                    deferred_out = (qq, och, po, W, my_tb, my_chunk,
                                    False, sq_pool)
                else:
                    out_stage(qq, och, po, W, my_tb, my_chunk, last, False)

                if not last:
                    if g == ngrp - 2:
                        pass       # merged into the final store below
                    else:
                        seng = nc.gpsimd if g < 6 else nc.sync
                        nch = (W + 511) // 512
                        pure_d = all(
                            COPY_SCHED[(my_chunk + i) % len(COPY_SCHED)] == 'd'
                            for i in range(nch))
                        if pure_d and not defer:
                            # all copies already emitted on DVE: store now
                            seng.dma_start(ot[:, c0:c0 + W], och[:, po:po + W])
                        else:
                            target = pending_store2 if defer else pending_store
                            target.append(
                                (seng, ot[:, c0:c0 + W], och[:, po:po + W]))
                else:
                    for dst, t1p, cw in pending_act:
                        nc.scalar.activation(dst, t1p[:, :cw], Ident,
                                             bias=g1s[:], scale=1.0)
                    pending_act = []
                    for eng, *dma_args in pending_store + pending_store2:
                        eng.dma_start(*dma_args)
                    pending_store = []
                    pending_store2 = []
                    pw = widths[g - 1]
                    nc.sync.dma_start(ot[:, c0 - pw:c0 + W],
                                      och[:, po - pw:po + W])
                c0 += W

    nc.compile()
    _PROG = nc
    return nc


def _pack_inputs(x: np.ndarray, W1: np.ndarray, W2: np.ndarray):
    assert x.shape == (B, DIM), x.shape
    assert W1.shape == (DIM, 32), W1.shape
    assert W2.shape == (32, 21), W2.shape
    G = _build_G(W1, W2)                      # fp64 (32, 35)
    Gm2 = (-2.0 * G).astype(np.float16)       # (32, 35)
    g1 = G.sum(axis=0)                        # (35,)

    g1s128 = np.empty(128, dtype=np.float64)
    for h in range(3):
        g1s128[35 * h:35 * h + 35] = g1
    g1s128[105:128] = g1[:23]

    blob = np.zeros((128, BW), dtype=np.float16)
    for gsub in range(SUB):
        blob[7 * gsub:7 * gsub + 7, BC_W1 + 32 * gsub:BC_W1 + 32 * gsub + 32] = \
            W1.astype(np.float16)
    for h in range(3):
        blob[32 * h:32 * h + 32, BC_GA + 35 * h:BC_GA + 35 * h + 35] = Gm2
    blob[96:128, BC_GA + 105:BC_GA + 128] = Gm2[:, :23]
    blob[96:128, BC_G2:BC_G2 + 12] = Gm2[:, 23:35]
    blob[0, BC_ONES:BC_ONES + 128] = 1.0
    blob[0, BC_G1T2:BC_G1T2 + T2_W] = np.tile(g1[23:35], T2_PER_TILE).astype(np.float16)

    # xt[m][7g+f, c] = x[m*B_CORE + g*COLS + c, f]
    xr = np.asarray(x, dtype=np.float16).reshape(NCORES, SUB, COLS, DIM)
    xt = np.ascontiguousarray(xr.transpose(0, 1, 3, 2).reshape(NCORES, K_IN, COLS))
    g1f = np.ascontiguousarray(g1s128[:, None], dtype=np.float32)
    in_maps = [{"xt": xt[m], "blob": blob, "g1f32": g1f} for m in range(NCORES)]
    return in_maps


def _unpack_outputs(results) -> np.ndarray:
    ot = np.stack([r["ot"] for r in results])       # (8, 128, 8960) f16
    A = ot[:, :, :COLS]                             # (8, 128, 8192)
    # subgroups 0..2: rows 35h+o
    a3 = A[:, :105, :].reshape(NCORES, 3, 35, COLS).transpose(0, 1, 3, 2)
    # subgroup 3 outputs 0..22: rows 105..127
    a4 = A[:, 105:128, :].transpose(0, 2, 1)        # (8, 8192, 23)
    # subgroup 3 outputs 23..34: T2 region [128 samples, 12] blocks
    idx = np.empty(NT2, dtype=np.int64)
    for _b in range(NT2):
        _t, _w = _t2_pos(_b)
        idx[_b] = 12 * (T2_STARTS[_t] + _w)
    cols = (idx[:, None] + np.arange(12)).reshape(-1)
    Bp = ot[:, :, COLS:][:, :, cols].reshape(NCORES, 128, NT2, 12)
    b4 = Bp.transpose(0, 2, 1, 3).reshape(NCORES, COLS, 12)
    out4 = np.concatenate([a4, b4], axis=2)         # (8, 8192, 35)
    out = np.concatenate([a3.reshape(NCORES, 3 * COLS, 35), out4], axis=1)
    return np.ascontiguousarray(out.reshape(B, 35), dtype=np.float32)


def run(x, W1, W2, **spmd_kwargs):
    """Run the kernel; returns (output, BassKernelResults)."""
    from concourse.bass_utils import run_bass_kernel_spmd
    nc = _get_prog()
    in_maps = _pack_inputs(np.asarray(x, dtype=np.float32),
                           np.asarray(W1, dtype=np.float32),
                           np.asarray(W2, dtype=np.float32))
    res = run_bass_kernel_spmd(nc, in_maps, core_ids=list(range(NCORES)), **spmd_kwargs)
    return _unpack_outputs(res.results), res


def kernel(x, W1, W2):
    out, _ = run(x, W1, W2)
    return out



# revision 2
# speedup vs baseline: 1.0123x; 1.0123x over previous
"""Trainium2 Bass kernel for nn_ExteriorDerivative (d of a 2-form via central FD).

Math: the reference's central finite difference collapses analytically:
  (x +/- eps e_d) @ W1 = z +/- eps*W1[d]  with z = x @ W1, and
  sin(z+a) - sin(z-a) = 2 cos(z) sin(a), so
  fd[d] = cos(z) @ (diag(sin(eps*W1[d])/eps) @ W2)
and the whole gather/sign/scatter pipeline folds into one (32, 35) matrix G:
  out = cos(x @ W1) @ G = g1 + (sin(z/2)^2) @ (-2 G),   g1 = G.sum(0)
using cos(z) = 1 - 2 sin^2(z/2)  (|z/2| < pi here, no range reduction).

Device pipeline per core (pure batch-parallel across 8 cores; 32768
samples/core packed 4 subgroups x 8192 columns so every elementwise tile
uses all 128 partitions; engine time in this regime is per *column*):
  mm1:  z = blockdiag(W1 x4)^T @ xt            [128, W] PSUM f32 (f16 mm)
  s   = Sin(0.5 z)                             ACT, PSUM->SBUF f16
  q   = s*s                                    DVE tensor_tensor (2x mode)
  mm2A: T1 = GA^T @ q                          [128, 512] PSUM
        GA [128,128] = blockdiag(-2G x3) plus rows 96:128 -> outputs 0:23
        of subgroup 3, so T1 rows are 3 full samples + 23/35 of a 4th.
  copy: och = T1 + g1 (per-partition f32 bias) DVE tensor_scalar / ACT Ident
        (GPSIMD cannot access PSUM, so copies split DVE/ACT only; ACT
        copies are deferred one group so they never stall the sin spine)
  mm2B: leftover 12 outputs of subgroup 3 via ldweights-stationary matmuls:
        T2[128 samp, 12] += q[96:128, blk]^T @ (-2G)[:, 23:35], with a
        rank-1 ones x g1[23:35] prefill matmul providing the bias.
  copy: o2 = T2 (pure DVE copy), DMA out       ot [128, 8960] f16
Timeline-shape choices (tuned against the TimelineSim cost model): group
widths 512 at the head/tail, 1024 mid; stores launch from Pool SWDGE early
and SP late (keeps SP.SEQ free for loads; avoids head-of-line blocking); a
t=0 dummy matmul starts the PE pstate-ramp clock so real matmuls run at
full rate; tiny first x chunk via Pool SWDGE overlaps the weight-blob load
on SP/HWDGE; input x shipped f16; weights in one f16 blob.
"""
import numpy as np
from itertools import combinations

DIM = 7
EPS = 1e-4
NCORES = 8
B = 262144
B_CORE = B // NCORES          # 32768
SUB = 4                       # subgroups stacked on partitions
COLS = B_CORE // SUB          # 8192 columns per core
K_IN = SUB * DIM              # 28 input partitions
GROUP = 1024                  # columns per z tile (2 psum banks)
PAIR = 2 * GROUP              # columns per s/q tile and per load/store
NPAIR = COLS // PAIR          # 4
NGRP = COLS // GROUP          # 8
T2_BLK = 128                  # samples per mm2B matmul (stationary free dim)
NT2 = COLS // T2_BLK          # 64 blocks
# T2 tile sizes in blocks (max 42 = 504 f32 cols per psum bank); a smaller
# final tile keeps the tail-critical copy short
T2_TILES = [32, 32]
T2_PER_TILE = max(T2_TILES)
T2_W = T2_PER_TILE * 12
assert sum(T2_TILES) == NT2
T2_STARTS = [sum(T2_TILES[:i]) for i in range(len(T2_TILES))]
def _t2_pos(tbi):
    for t in range(len(T2_TILES) - 1, -1, -1):
        if tbi >= T2_STARTS[t]:
            return t, tbi - T2_STARTS[t]
OUT_W = COLS + NT2 * 12       # 8960 output columns

# blob column layout (f16, [128, BW])
BC_W1 = 0                     # [0:28, 0:128]     blockdiag W1 x4
BC_GA = 128                   # [0:128, 128:256]  GA
BC_G2 = 256                   # [96:128, 256:268] (-2G)[:,23:35], partitions 96..127
BC_ONES = 268                 # [0:1, 268:396]    ones row (prefill lhsT)
BC_G1T2 = 396                 # [0:1, ...]        g1[23:35] tiled (prefill rhs)
BW = 396 + T2_W

# T1-copy engine schedule (16 chunks of 512): 'd' DVE / 'a' ACT (deferred)
COPY_SCHED = "dddaadddaddaadda"
POOL_SQ = set()               # Pool squares always lose (measured); keep empty

# ---- static exterior-derivative index maps (mirrors reference.py) ----
_IDX3 = list(combinations(range(DIM), 3))
_POS2 = {t: i for i, t in enumerate(combinations(range(DIM), 2))}
_D2 = []
for _out, (i, j, k) in enumerate(_IDX3):
    for _p, (a, b, c) in enumerate([(i, j, k), (j, i, k), (k, i, j)]):
        bc = tuple(sorted((b, c)))
        s = (-1) ** _p * (1 if (b, c) == bc else -1)
        _D2.append((_out, _POS2[bc], a, s))


def _build_G(W1: np.ndarray, W2: np.ndarray) -> np.ndarray:
    """G[j, o] = sum_t SIGNS[t] * sin(EPS*W1[DCOORD[t], j])/EPS * W2[j, IN_POS[t]]  (fp64)."""
    W1d = W1.astype(np.float64)
    W2d = W2.astype(np.float64)
    G = np.zeros((32, 35), dtype=np.float64)
    for out_pos, in_pos, dcoord, sign in _D2:
        G[:, out_pos] += sign * (np.sin(EPS * W1d[dcoord, :]) / EPS) * W2d[:, in_pos]
    return G


_PROG = None


def _get_prog():
    global _PROG
    if _PROG is not None:
        return _PROG
    import concourse.bacc as bacc
    import concourse.bass as bass
    import concourse.tile as tile
    import concourse.mybir as mybir
    from concourse.alu_op_type import AluOpType as Alu

    F32 = mybir.dt.float32
    F16 = mybir.dt.float16
    Sin = mybir.ActivationFunctionType.Sin
    Ident = mybir.ActivationFunctionType.Identity

    nc = bacc.Bacc("TRN2", target_bir_lowering=False, debug=False, num_devices=NCORES)
    xt = nc.dram_tensor("xt", [K_IN, COLS], F16, kind="ExternalInput")
    blob = nc.dram_tensor("blob", [128, BW], F16, kind="ExternalInput")
    g1f32 = nc.dram_tensor("g1f32", [128, 1], F32, kind="ExternalInput")
    ot = nc.dram_tensor("ot", [128, OUT_W], F16, kind="ExternalOutput")

    with tile.TileContext(nc) as tc:
        with (
            tc.tile_pool(name="singles", bufs=1) as singles,
            tc.tile_pool(name="xin", bufs=2) as xpool,
            tc.tile_pool(name="och", bufs=2) as opool,
            tc.tile_pool(name="o2ch", bufs=2) as o2pool,
            tc.tile_pool(name="ssp", bufs=2) as spool,
            tc.tile_pool(name="qqp", bufs=2) as qpool,
            tc.tile_pool(name="zps", bufs=2, space=bass.MemorySpace.PSUM) as zpsum,
            tc.tile_pool(name="t1ps", bufs=3, space=bass.MemorySpace.PSUM) as t1psum,
            tc.tile_pool(name="t2ps", bufs=1, space=bass.MemorySpace.PSUM) as t2psum,
        ):
            warm = singles.tile([1, 64], F16)
            nc.vector.memset(warm[:], 0.0)
            bl = singles.tile([128, BW], F16)
            nc.sync.dma_start(bl[0:K_IN, 0:128], blob[0:K_IN, 0:128])
            g1s = singles.tile([128, 1], F32)

            w1_ap = bl[0:K_IN, BC_W1:BC_W1 + 128]
            ga_ap = bl[0:128, BC_GA:BC_GA + 128]
            g2_ap = bl[96:128, BC_G2:BC_G2 + 12]
            ones_ap = bl[0:1, BC_ONES:BC_ONES + 128]
            g1t2_ap = bl[0:1, BC_G1T2:BC_G1T2 + T2_W]

            wps = t1psum.tile([128, 512], F32, tag="t1")
            nc.tensor.matmul(wps[0:1, 0:64], warm[0:1, 0:1], warm[0:1, 0:64])

            # group widths: short head groups (fast pipeline fill) and short
            # tail groups (short drain chain)
            widths = [512, 512, 1024, 1024, 1024, 1024, 1024, 1024, 512, 512]
            assert sum(widths) == COLS
            t2 = None
            tb = 0               # T2 block counter
            chunk = 0            # T1-copy chunk counter (16 total)
            pending_act = []     # deferred ACT copies (dst, t1) from prior group
            pending_store = []   # stores deferred until the prior group's
                                 # copies (incl. deferred ACT ones) are emitted
            pending_store2 = []  # two groups late (pool-squared groups)
            deferred_out = None  # out-stage of a Pool-squared group
            xins = {}
            ochs = {}
            sss = {}
            qqs = {}
            ngrp = len(widths)

            # leftover 12 outputs of subgroup 3 (q-stationary matmuls);
            # block -> tile mapping is fixed by the block index; accumulation
            # order within a tile is free (disjoint 12-col ranges)
            def t2_blocks(qq, po, W, tb0):
                nonlocal t2
                for b in range(W // T2_BLK):
                    tbi = tb0 + b
                    t, w = _t2_pos(tbi)
                    tw = T2_TILES[t] * 12
                    if w == 0:
                        t2 = t2psum.tile([128, T2_W], F32, tag="t2")
                        nc.tensor.matmul(t2[:, :tw], ones_ap, g1t2_ap[:, :tw],
                                         start=True, stop=False)
                    qb = qq[96:128, po + b * T2_BLK:po + (b + 1) * T2_BLK]
                    nc.tensor.matmul(t2[:, 12 * w:12 * w + 12], qb, g2_ap,
                                     start=False, stop=(w == T2_TILES[t] - 1),
                                     tile_position=(96, 0))
                    if w == T2_TILES[t] - 1:
                        o2 = o2pool.tile([128, T2_W], F16, tag="o2")
                        t2o = COLS + 12 * T2_STARTS[t]
                        nc.vector.tensor_copy(o2[:, :tw], t2[:, :tw])
                        (nc.sync if tbi == NT2 - 1 else nc.gpsimd
                         ).dma_start(ot[:, t2o:t2o + tw], o2[:, :tw])

            # T1 copies (mm2A + psum->sbuf+bias) then the T2 side stream
            def out_stage(qq, och, po, W, tb0, chunk0, last, force_d):
                for i, s in enumerate(range(0, W, 512)):
                    cw = min(512, W - s)
                    t1 = t1psum.tile([128, 512], F32, tag="t1")
                    nc.tensor.matmul(t1[:, :cw], ga_ap,
                                     qq[:, po + s:po + s + cw])
                    dst = och[:, po + s:po + s + cw]
                    e = 'd' if force_d else COPY_SCHED[(chunk0 + i)
                                                       % len(COPY_SCHED)]
                    if e == 'a' and not last:
                        pending_act.append((dst, t1, cw))
                    elif e == 'a':
                        nc.scalar.activation(dst, t1[:, :cw], Ident,
                                             bias=g1s[:], scale=1.0)
                    else:
                        nc.vector.tensor_scalar(dst, t1[:, :cw], g1s[:], None,
                                                Alu.add)
                t2_blocks(qq, po, W, tb0)

            c0 = 0
            for g, W in enumerate(widths):
                pi = c0 // PAIR
                po = c0 - pi * PAIR
                if pi not in xins:
                    xin = xpool.tile([K_IN, PAIR], F16, tag="xin")
                    if pi == 0:
                        # tiny first chunk via Pool SWDGE overlapping the
                        # w1 load on SP/HWDGE -> earliest possible mm1;
                        # weight-blob remainder + bias column follow the
                        # second x chunk so they don't delay sin(g1)
                        nc.gpsimd.dma_start(xin[:, :512], xt[:, :512])
                        nc.sync.dma_start(xin[:, 512:], xt[:, 512:PAIR])
                        nc.sync.dma_start(bl[:, 128:], blob[:, 128:])
                        nc.sync.dma_start(g1s[:], g1f32[:])
                    else:
                        nc.sync.dma_start(xin[:], xt[:, pi * PAIR:(pi + 1) * PAIR])
                    xins[pi] = xin
                    och = opool.tile([128, PAIR], F16, tag="och")
                    ss = spool.tile([128, PAIR], F16, tag="ss")
                    qq = qpool.tile([128, PAIR], F16, tag="qq")
                    ochs[pi], sss[pi], qqs[pi] = och, ss, qq
                xin, och, ss, qq = xins[pi], ochs[pi], sss[pi], qqs[pi]
                last = g == ngrp - 1

                zp = zpsum.tile([128, GROUP], F32, tag="zp")
                for s in range(0, W, 512):
                    cw = min(512, W - s)
                    nc.tensor.matmul(zp[:, s:s + cw], w1_ap,
                                     xin[:, po + s:po + s + cw])
                nc.scalar.activation(ss[:, po:po + W], zp[:, :W], Sin,
                                     bias=0.0, scale=0.5)
                # deferred ACT copies sit after this sin in the ACT queue, so
                # their mm2A dependency is long satisfied -> no ACT stall
                for dst, t1p, cw in pending_act:
                    nc.scalar.activation(dst, t1p[:, :cw], Ident,
                                         bias=g1s[:], scale=1.0)
                pending_act = []
                for eng, *dma_args in pending_store:
                    eng.dma_start(*dma_args)
                pending_store = pending_store2
                pending_store2 = []
                sq_pool = (g in POOL_SQ and not last
                           and g + 1 not in POOL_SQ and g >= 1)
                defer = sq_pool and not last
                (nc.gpsimd if sq_pool else nc.vector).tensor_tensor(
                    qq[:, po:po + W], ss[:, po:po + W], ss[:, po:po + W],
                    Alu.mult)

                my_tb, my_chunk = tb, chunk
                tb += W // T2_BLK
                chunk += (W + 511) // 512
                if deferred_out is not None:
                    out_stage(*deferred_out)
                    deferred_out = None
                if defer:
                    # software-pipelined emission: this group's out-stage is
                    # emitted after the NEXT group's mm1/sin/sq, so mm1(g+1)
                    # never queues behind mm2A/T2 matmuls in the in-order PE
                    # stream and the copies' mm2A deps are met on arrival
                    deferred_out = (qq, och, po, W, my_tb, my_chunk,
                                    False, sq_pool)
                else:
                    out_stage(qq, och, po, W, my_tb, my_chunk, last, False)

                if not last:
                    if g == ngrp - 2:
                        pass       # merged into the final store below
                    else:
                        seng = nc.gpsimd if g < 1 else nc.sync
                        nch = (W + 511) // 512
                        pure_d = all(
                            COPY_SCHED[(my_chunk + i) % len(COPY_SCHED)] == 'd'
                            for i in range(nch))
                        if pure_d and not defer:
                            # all copies already emitted on DVE: store now
                            seng.dma_start(ot[:, c0:c0 + W], och[:, po:po + W])
                        else:
                            target = pending_store2 if defer else pending_store
                            target.append(
                                (seng, ot[:, c0:c0 + W], och[:, po:po + W]))
                else:
                    for dst, t1p, cw in pending_act:
                        nc.scalar.activation(dst, t1p[:, :cw], Ident,
                                             bias=g1s[:], scale=1.0)
                    pending_act = []
                    for eng, *dma_args in pending_store + pending_store2:
                        eng.dma_start(*dma_args)
                    pending_store = []
                    pending_store2 = []
                    pw = widths[g - 1]
                    nc.sync.dma_start(ot[:, c0 - pw:c0 + W],
                                      och[:, po - pw:po + W])
                c0 += W

    nc.compile()
    _PROG = nc
    return nc


def _pack_inputs(x: np.ndarray, W1: np.ndarray, W2: np.ndarray):
    assert x.shape == (B, DIM), x.shape
    assert W1.shape == (DIM, 32), W1.shape
    assert W2.shape == (32, 21), W2.shape
    G = _build_G(W1, W2)                      # fp64 (32, 35)
    Gm2 = (-2.0 * G).astype(np.float16)       # (32, 35)
    g1 = G.sum(axis=0)                        # (35,)

    g1s128 = np.empty(128, dtype=np.float64)
    for h in range(3):
        g1s128[35 * h:35 * h + 35] = g1
    g1s128[105:128] = g1[:23]

    blob = np.zeros((128, BW), dtype=np.float16)
    for gsub in range(SUB):
        blob[7 * gsub:7 * gsub + 7, BC_W1 + 32 * gsub:BC_W1 + 32 * gsub + 32] = \
            W1.astype(np.float16)
    for h in range(3):
        blob[32 * h:32 * h + 32, BC_GA + 35 * h:BC_GA + 35 * h + 35] = Gm2
    blob[96:128, BC_GA + 105:BC_GA + 128] = Gm2[:, :23]
    blob[96:128, BC_G2:BC_G2 + 12] = Gm2[:, 23:35]
    blob[0, BC_ONES:BC_ONES + 128] = 1.0
    blob[0, BC_G1T2:BC_G1T2 + T2_W] = np.tile(g1[23:35], T2_PER_TILE).astype(np.float16)

    # xt[m][7g+f, c] = x[m*B_CORE + g*COLS + c, f]
    xr = np.asarray(x, dtype=np.float16).reshape(NCORES, SUB, COLS, DIM)
    xt = np.ascontiguousarray(xr.transpose(0, 1, 3, 2).reshape(NCORES, K_IN, COLS))
    g1f = np.ascontiguousarray(g1s128[:, None], dtype=np.float32)
    in_maps = [{"xt": xt[m], "blob": blob, "g1f32": g1f} for m in range(NCORES)]
    return in_maps


def _unpack_outputs(results) -> np.ndarray:
    ot = np.stack([r["ot"] for r in results])       # (8, 128, 8960) f16
    A = ot[:, :, :COLS]                             # (8, 128, 8192)
    # subgroups 0..2: rows 35h+o
    a3 = A[:, :105, :].reshape(NCORES, 3, 35, COLS).transpose(0, 1, 3, 2)
    # subgroup 3 outputs 0..22: rows 105..127
    a4 = A[:, 105:128, :].transpose(0, 2, 1)        # (8, 8192, 23)
    # subgroup 3 outputs 23..34: T2 region [128 samples, 12] blocks
    idx = np.empty(NT2, dtype=np.int64)
    for _b in range(NT2):
        _t, _w = _t2_pos(_b)
        idx[_b] = 12 * (T2_STARTS[_t] + _w)
    cols = (idx[:, None] + np.arange(12)).reshape(-1)
    Bp = ot[:, :, COLS:][:, :, cols].reshape(NCORES, 128, NT2, 12)
    b4 = Bp.transpose(0, 2, 1, 3).reshape(NCORES, COLS, 12)
    out4 = np.concatenate([a4, b4], axis=2)         # (8, 8192, 35)
    out = np.concatenate([a3.reshape(NCORES, 3 * COLS, 35), out4], axis=1)
    return np.ascontiguousarray(out.reshape(B, 35), dtype=np.float32)


def run(x, W1, W2, **spmd_kwargs):
    """Run the kernel; returns (output, BassKernelResults)."""
    from concourse.bass_utils import run_bass_kernel_spmd
    nc = _get_prog()
    in_maps = _pack_inputs(np.asarray(x, dtype=np.float32),
                           np.asarray(W1, dtype=np.float32),
                           np.asarray(W2, dtype=np.float32))
    res = run_bass_kernel_spmd(nc, in_maps, core_ids=list(range(NCORES)), **spmd_kwargs)
    return _unpack_outputs(res.results), res


def kernel(x, W1, W2):
    out, _ = run(x, W1, W2)
    return out

